# revision 28
# baseline (speedup 1.0000x reference)
"""Binarized 3x3 conv (BinarizeConv2dSDP) on 8 Trainium2 NeuronCores.

out = conv2d(sign(x), sign(M), pad=1) * alpha
  x: [32, 256, 56, 56] f32, M: [256, 256, 3, 3] f32, alpha: [256, 1, 1] f32

Strategy (data-parallel over batch, 4 images per core, identical SPMD program):
  - Weights are binarized + transposed + packed to fp8 on the host (they are
    replicated anyway); the device just DMAs the 576KB block.
  - The padded fp8 image lives in PER-ROW-BLOCK tiles [128, 2(cin hi), 10x57]
    with a 2-row halo. Tile dependencies are tile-granular, so this is what
    lets a row block's matmul chain start as soon as its own rows are DMA'd
    and signed, instead of gating on the whole image.
  - Rows are 57 wide with one zero column at index 0 of each row; a row's
    left pad doubles as the previous row's right pad, so the matmul free dim
    is 456 (8 rows x 57) and all pad zeros in a tile are one strided memset.
  - A burst of warm-up matmuls keeps the PE HAM busy through the head so the
    real chains run at the full 2.4 GHz clock from the start.
  - 9 taps x (4 img x 7 row-blocks x 2 cout-halves) DoubleRow fp8 matmuls,
    each contracting all 256 cin at once, accumulated in PSUM. All values
    are +-1/0 so fp8 math is exact.
  - Drain PSUM through DVE tensor_scalar mul by per-channel alpha into
    2-row-block output buffers; output DMAs are issued from the Scalar
    HWDGE ring (separate from the Sync input ring). The last image stores
    per row block to shorten the tail.
"""

import os
import sys
import types

import ml_dtypes
import numpy as np

# ---- problem constants (hardcoded per contract) ----
N, CIN, COUT, H, W = 32, 256, 256, 56, 56
NCORES = 8
NSH = N // NCORES  # images per core = 4
S = W + 1  # 57: row stride; col 0 of each row is the zero pad
NHB = 7  # row blocks of 8 output rows
IMGH = 640  # per-half flat stride (>= 572 used), 16-aligned
FD = 8 * S - 1  # 455: 8 padded rows minus the trailing garbage elem
NDUMMY = 40  # PE warm-up matmuls issued while the first rows stream in
LEAD_DMA = 4  # row-block chunks to keep in flight ahead of the chains
LEAD_SIGN = 2

_BUILT = {}
LAST_EXEC_NS = None
LAST_TRACE = None


def _build():
    import concourse.bass as bass
    import concourse.mybir as mybir
    import concourse.tile as tile
    from concourse.bass import ds

    fp8 = mybir.dt.float8e4
    f32 = mybir.dt.float32
    bf16 = mybir.dt.bfloat16

    nc = bass.Bass(name="binconv")
    # x ships as bf16: sign-exact for any non-subnormal input, half the DMA
    x_d = nc.dram_tensor("x", [NSH, CIN, H, W], bf16, kind="ExternalInput")
    w_d = nc.dram_tensor("W", [128, 9, 2, 2, 128], fp8, kind="ExternalInput")
    a_d = nc.dram_tensor("alpha", [COUT, 1, 1], f32, kind="ExternalInput")
    o_d = nc.dram_tensor("out", [NSH, COUT, H, W], f32, kind="ExternalOutput")

    # per row block hb: padded-image rows 8hb..8hb+9 live in local rows 0..9;
    # local row L holds real image row 8hb+L-1 (rows -1 and 56 are zero pads)
    def src_rows(hb):
        lo = max(0, 8 * hb - 1)
        hi = min(H - 1, 8 * hb + 8)
        return lo, hi - lo + 1, lo + 1 - 8 * hb  # r0, nr, local row of r0

    with tile.TileContext(nc) as tc:
        with (
            tc.tile_pool(name="consts", bufs=1) as consts,
            tc.tile_pool(name="xin", bufs=12) as xin_pool,
            tc.tile_pool(name="xpad", bufs=NSH * NHB) as xpad_pool,
            tc.tile_pool(name="osb", bufs=6) as osb_pool,
            tc.tile_pool(name="psum", bufs=8, space="PSUM") as psum_pool,
        ):
            # ---- constants ----
            # weights split across three tiles, uploaded on three different
            # DMA queues, so each tap group lands just before chain 0 needs it
            w_sba = consts.tile([128, 2, 2, 2, 128], fp8, tag="wsba")
            w_sbb = consts.tile([128, 3, 2, 2, 128], fp8, tag="wsbb")
            w_sbc = consts.tile([128, 4, 2, 2, 128], fp8, tag="wsbc")

            def wslice(t, co):
                if t < 2:
                    return w_sba[:, t, :, co, :]
                if t < 5:
                    return w_sbb[:, t - 2, :, co, :]
                return w_sbc[:, t - 5, :, co, :]
            alpha_sb = consts.tile([128, 2], f32, tag="alpha")
            dummy_sb = consts.tile([128, 128], fp8, tag="dmy")
            dsign_f = consts.tile([128, 8], f32, tag="dsf")
            dsign_o = consts.tile([128, 8], fp8, tag="dso")
            nc.gpsimd.memset(dummy_sb[:], 0.0)
            # taps 5-8 ride the GpSimd SWDGE queue (3rd DMA path); they are
            # needed last, which absorbs SWDGE's ~2us completion latency
            nc.gpsimd.dma_start(w_sbc[:], w_d[:, 5:9])
            nc.gpsimd.memset(dsign_f[:], 0.0)
            # warm-up sign: hoists the lazy ACT_TABLE_LOAD (1.3us) off the
            # first real sign's critical path
            nc.scalar.sign(dsign_o[:], dsign_f[:])

            # ---- PE warm-up: keep HAM busy so real matmuls start at 2.4GHz
            dps = psum_pool.tile([128, 128], f32, tag="ps", name="dummy")
            for _ in range(NDUMMY):
                nc.tensor.matmul(
                    dps[:], dummy_sb[:], dummy_sb[:],
                    start=True, stop=True, skip_group_check=True,
                )

            xi = {}
            xpt = {}

            def issue_in_dma(k, split=False):
                n, hb = k // NHB, k % NHB
                r0, nr, _ = src_rows(hb)
                t = xin_pool.tile([128, 2, 10, W], bf16, tag="xi", name=f"xi{n}{hb}")
                xi[k] = t
                if split:
                    # halves on the two HWDGE rings -> parallel transfer
                    for j, eng in ((0, nc.sync), (1, nc.scalar)):
                        eng.dma_start(
                            t[:, j, 0:nr, :],
                            x_d[n, j * 128 : (j + 1) * 128, r0 : r0 + nr, :],
                        )
                else:
                    # one DMA, both cin halves: partition p, half j <- chan j*128+p
                    src = x_d[n].rearrange("(j p) h w -> p j h w", j=2)
                    nc.sync.dma_start(t[:, :, 0:nr, :], src[:, :, r0 : r0 + nr, :])

            def issue_pads(k):
                n, hb = k // NHB, k % NHB
                t = xpad_pool.tile([128, 2, IMGH], fp8, tag="xp", name=f"xp{n}{hb}")
                xpt[k] = t
                for j in range(2):
                    rv = t[:, j, ds(0, 10 * S)].rearrange("p (r c) -> p r c", c=S)
                    nc.gpsimd.memset(rv[:, :, 0:1], 0.0)  # pad col of rows 0..9
                    nc.gpsimd.memset(t[:, j, ds(10 * S, IMGH - 10 * S)], 0.0)
                    if hb == 0:
                        nc.gpsimd.memset(t[:, j, ds(0, S)], 0.0)  # top pad row
                    if hb == NHB - 1:
                        nc.gpsimd.memset(t[:, j, ds(9 * S, S)], 0.0)  # bottom pad

            def issue_sign(k):
                r0, nr, l0 = src_rows(k % NHB)
                t = xpt[k]
                rv = t[:, :, ds(0, 10 * S)].rearrange("p j (r c) -> p j r c", c=S)
                nc.scalar.sign(
                    rv[:, :, l0 : l0 + nr, 1:57], xi[k][:, :, 0:nr, :]
                )

            # head order (each HWDGE ring transfers strictly in order): the
            # first two row-block chunks are j-split across both rings so
            # they land fastest; weights interleave on the sync ring
            for k in range(NSH * NHB):
                issue_pads(k)
            issue_in_dma(0, split=True)
            nc.sync.dma_start(w_sba[:], w_d[:, 0:2])
            nc.scalar.dma_start(w_sbb[:], w_d[:, 2:5])
            issue_in_dma(1, split=True)
            # the first signs go on the scalar queue BEFORE the ring-gated
            # later chunk issues, or they'd wait behind those issues' waits
            issue_sign(0)
            issue_sign(1)
            issue_in_dma(2, split=True)
            issue_in_dma(3, split=True)
            for co in range(2):
                nc.sync.dma_start(
                    alpha_sb[:, co : co + 1],
                    a_d[co * 128 : (co + 1) * 128, 0, :],
                )

            # ---- main: 9-tap DoubleRow chains per (img, couthalf, rowblock)
            osb_cur = {}  # co -> (tile, base_hb)
            for k in range(NSH * NHB):
                n, hb = k // NHB, k % NHB
                if k + LEAD_DMA < NSH * NHB:
                    issue_in_dma(k + LEAD_DMA)
                if k + LEAD_SIGN < NSH * NHB:
                    issue_sign(k + LEAD_SIGN)
                for co in range(2):
                    acc = psum_pool.tile([128, 8 * S], f32, tag="ps", name="acc")
                    accv = acc[:].rearrange("p (r c) -> p r c", c=S)
                    for t in range(9):
                        dy, dx = t // 3, t % 3
                        nc.tensor.matmul(
                            acc[:, 0:FD],
                            wslice(t, co),
                            xpt[k][:, :, ds(dy * S + dx, FD)],
                            start=(t == 0),
                            stop=(t == 8),
                            perf_mode=mybir.MatmulPerfMode.DoubleRow,
                            skip_group_check=True,
                        )
                    # drain: alpha scale into a 2-row-block store buffer
                    # (last image stores per row block to shorten the tail)
                    batch = 1 if n == NSH - 1 else 2
                    cur = osb_cur.get(co)
                    if cur is None or hb - cur[1] >= batch or hb == 0:
                        ob = osb_pool.tile(
                            [128, batch * 8, W], f32, tag="ob", name=f"ob{co}"
                        )
                        osb_cur[co] = cur = (ob, hb)
                    ob, hb0 = cur
                    last = n == NSH - 1 and co == 1 and hb == NHB - 1
                    if last:
                        # split the final drain/store in two to overlap the
                        # DVE drain with the store DMA at the kernel tail
                        for h in range(2):
                            nc.vector.tensor_scalar_mul(
                                ob[:, 4 * h : 4 * h + 4, :],
                                accv[:, 4 * h : 4 * h + 4, 0:56],
                                alpha_sb[:, co : co + 1],
                            )
                            nc.scalar.dma_start(
                                o_d[
                                    n,
                                    co * 128 : (co + 1) * 128,
                                    8 * hb + 4 * h : 8 * hb + 4 * h + 4,
                                ],
                                ob[:, 4 * h : 4 * h + 4, :],
                            )
                        osb_cur[co] = None
                        continue
                    nc.vector.tensor_scalar_mul(
                        ob[:, (hb - hb0) * 8 : (hb - hb0) * 8 + 8, :],
                        accv[:, :, 0:56],
                        alpha_sb[:, co : co + 1],
                    )
                    if hb - hb0 == batch - 1 or hb == NHB - 1:
                        nrows = (hb - hb0 + 1) * 8
                        nc.scalar.dma_start(
                            o_d[
                                n,
                                co * 128 : (co + 1) * 128,
                                8 * hb0 : 8 * hb0 + nrows,
                            ],
                            ob[:, 0:nrows, :],
                        )
                        osb_cur[co] = None
    return nc


def _pack_weights(M):
    """sign(M) -> fp8 lhsT layout [cin_lo, tap, cin_hi, cout_hi, cout_lo]."""
    s = np.sign(np.ascontiguousarray(M, dtype=np.float32))
    # [co, m, j, p, ty, tx] -> [p, ty, tx, j, co, m]
    s = s.reshape(2, 128, 2, 128, 3, 3).transpose(3, 4, 5, 2, 0, 1)
    return np.ascontiguousarray(s.reshape(128, 9, 2, 2, 128)).astype(
        ml_dtypes.float8_e4m3
    )


def _install_compat():
    """Environment shims (inlined so kernel.py is self-contained).

    1. `antenv.axon_hooks` is missing from this image; provide it so
       `run_bass_kernel_spmd(trace=True)` can capture NTFF profiles.
    2. The walrus build rejects >1 sync-wait on the NOP/Drain control
       struct; TileContext's tail drain aggregates one wait per outstanding
       semaphore. Patch `_drain_and_barrier` to spread the waits over a
       chain of SP nops (1 wait each) before the drain.
    """
    if "antenv.axon_hooks" not in sys.modules:
        try:
            import antenv

            mod = types.ModuleType("antenv.axon_hooks")
            _hook = [None]

            def set_axon_ntff_profile_hook(h):
                _hook[0] = h

            def get_axon_ntff_profile_hook():
                if _hook[0] is None:
                    try:
                        from trn_agent_boot.trn_boot import _ntff_profile_via_ctypes

                        _hook[0] = _ntff_profile_via_ctypes(
                            "/opt/axon/libaxon_pjrt.so"
                        )
                    except Exception:
                        return None
                return _hook[0]

            mod.set_axon_ntff_profile_hook = set_axon_ntff_profile_hook
            mod.get_axon_ntff_profile_hook = get_axon_ntff_profile_hook
            sys.modules["antenv.axon_hooks"] = mod
            antenv.axon_hooks = mod
        except ImportError:
            pass

    import json as _json

    from concourse import bass2jax, bass_utils

    if getattr(bass_utils, "_wait_split_patched", False):
        return

    _orig_compile = bass_utils.compile_bir_kernel

    def _split_waits(bir_json: bytes, limit: int = 1) -> bytes:
        m = _json.loads(bir_json)
        changed = False
        for fn in m.get("functions", []):
            for blk in fn.get("blocks", []):
                new = []
                for inst in blk.get("instructions", []):
                    si = inst.get("sync_info") or {}
                    waits = si.get("on_wait") or []
                    eng = inst.get("engine")
                    if len(waits) > limit and eng:
                        excess = waits[: len(waits) - limit]
                        for k in range(0, len(excess), limit):
                            new.append(
                                {
                                    "debug": inst.get("debug", 0),
                                    "engine": eng,
                                    "ins": [],
                                    "name": f"{inst['name']}-w{k}",
                                    "opcode": "NoOp",
                                    "outs": [],
                                    "sync_info": {
                                        "on_wait": excess[k : k + limit],
                                        "on_update": [],
                                    },
                                }
                            )
                        si = dict(si)
                        si["on_wait"] = waits[len(waits) - limit :]
                        inst = dict(inst)
                        inst["sync_info"] = si
                        changed = True
                    new.append(inst)
                blk["instructions"] = new
        if not changed:
            return bir_json
        return _json.dumps(m).encode()

    def _patched_compile(bir_json, tmpdir, neff_name="file.neff"):
        return _orig_compile(_split_waits(bir_json), tmpdir, neff_name)

    bass_utils.compile_bir_kernel = _patched_compile
    bass2jax.compile_bir_kernel = _patched_compile
    bass_utils._wait_split_patched = True

    maxsem = os.environ.get("BINCONV_MAXSEM")
    if maxsem:
        import stat
        import tempfile

        real = bass_utils.get_walrus_driver()
        wrap = os.path.join(tempfile.mkdtemp(), "walrus_wrap.sh")
        with open(wrap, "w") as f:
            f.write(f'#!/bin/sh\nexec "{real}" "$@" --max-sem-num={maxsem}\n')
        os.chmod(wrap, os.stat(wrap).st_mode | stat.S_IEXEC)
        bass_utils.get_walrus_driver = lambda: wrap


def _get_nc():
    if "nc" not in _BUILT:
        _install_compat()
        _BUILT["nc"] = _build()
    return _BUILT["nc"]


def kernel(x, M, alpha):
    global LAST_EXEC_NS, LAST_TRACE
    from concourse import bass_utils

    nc = _get_nc()
    x = np.ascontiguousarray(x, dtype=np.float32).astype(ml_dtypes.bfloat16)
    alpha = np.ascontiguousarray(alpha, dtype=np.float32)
    Wp = _pack_weights(M)
    in_maps = [
        {"x": x[i * NSH : (i + 1) * NSH], "W": Wp, "alpha": alpha}
        for i in range(NCORES)
    ]
    trace = bool(int(os.environ.get("BINCONV_TRACE", "0")))
    res = bass_utils.run_bass_kernel_spmd(
        nc, in_maps, core_ids=list(range(NCORES)), trace=trace
    )
    LAST_EXEC_NS = res.exec_time_ns
    LAST_TRACE = res.instructions_and_trace[1] if res.instructions_and_trace else None
    return np.concatenate([r["out"] for r in res.results], axis=0)
